# revision 20
# baseline (speedup 1.0000x reference)
"""MoE update-MLP Trainium2 kernel (8-core SPMD, data-parallel over pixels).

Problem: x (4,192,128,128); a per-pixel router picks top-2 of 8 experts; each
expert is a 3-layer 1x1-conv MLP (192->384 gelu ->384 gelu ->192); output is
the gate-weighted sum over experts.

Sharding: H=128 split into 8 chunks of 16 rows; each core handles
4*16*128 = 8192 pixels and computes all 8 experts densely (gates of
non-top-2 experts are exactly 0, so dense gate-weighted accumulation is
exact, and per-pixel dynamic routing/gather is avoided).

Per 512-pixel tile on each core:
 - router logits computed transposed ([128 pix, 8 experts]) via K=8 fp32
   matmuls (fp32 so top-2 ranking matches the fp32 reference bit-for-bit)
 - top-2 + 2-way softmax via masked-max + is_equal on DVE and a
   tanh-based sigmoid on ACT (gelu_and_others table has Gelu+Tanh, so a
   single activation-table load covers the whole kernel)
 - gates transposed back with 4 PE transposes into one PSUM tile; each
   expert's gate row broadcast to [128, 512] on GpSimd with
   partition_broadcast (keeps those rows off the saturated PE array)
 - per expert: L1 (2x3 matmuls, contraction zero-padded 192->256 to keep
   K=128), exact Gelu+per-partition bias on ACT, L2 (3x3), Gelu+bias,
   per-pixel gate multiply on DVE, L3 accumulated over all 8 experts in
   PSUM; the b3 bias enters as one K=8-padded matmul against the gate
   rows (sum_e g_e*b3_e); result copied to SBUF on DVE and DMA'd out.

All matmul operands are float32r (full-rate on the PE at free-dim 512,
~1e-4 rounding). Weights are staged per-expert in SBUF tiles (one
contiguous DMA each, issued from GpSimd so the Sync sequencer's serial
descriptor generation doesn't delay the first tile's x/r loads).
"""

import ml_dtypes
import numpy as np

import concourse.bacc as bacc
import concourse.mybir as mybir
import concourse.tile as tile
from concourse.bass_utils import run_bass_kernel_spmd
from concourse.masks import make_identity

F32 = mybir.dt.float32
F32R = mybir.dt.float32r
FP8 = mybir.dt.float8e4
DR = mybir.MatmulPerfMode.DoubleRow
AF = mybir.ActivationFunctionType
ALU = mybir.AluOpType

N_CORES = 8
B, IN_C, H, W = 4, 192, 128, 128
R_C, E, HID, OUT_C = 8, 8, 384, 192
HS = H // N_CORES            # 16 rows of H per core
PIX_B = HS * W               # 2048 pixels per batch image per core
TILE = 512                   # pixels per compute tile
NT_B = PIX_B // TILE         # 4 tiles per batch image

_nc_cache: dict = {}


def _build(act: str = "gelu", compile: bool = True):
    """Build the (SPMD-identical) Bass program for one core."""
    nc = bacc.Bacc("TRN2", target_bir_lowering=False, debug=False)

    x8h_in = nc.declare_dram_parameter("x8h", [B, 96, 2, PIX_B], FP8, isOutput=False)
    x8l_in = nc.declare_dram_parameter("x8l", [B, 96, 2, PIX_B], FP8, isOutput=False)
    r_in = nc.declare_dram_parameter("r", [B, R_C, PIX_B], F32, isOutput=False)
    w1h_in = nc.declare_dram_parameter("w1h", [E, 96, 2, HID], FP8, isOutput=False)
    w1l_in = nc.declare_dram_parameter("w1l", [E, 96, 2, HID], FP8, isOutput=False)
    w2_in = nc.declare_dram_parameter("w2t", [E, 128, 3, HID], F32R, isOutput=False)
    w3_in = nc.declare_dram_parameter("w3t", [E, 128, 3, OUT_C], F32R, isOutput=False)
    rwt_in = nc.declare_dram_parameter("rwt", [R_C, E], F32, isOutput=False)
    rb_in = nc.declare_dram_parameter("rb", [128, E], F32, isOutput=False)
    b1_in = nc.declare_dram_parameter("b1t", [128, E * 3], F32, isOutput=False)
    b2_in = nc.declare_dram_parameter("b2t", [128, E * 3], F32, isOutput=False)
    b3_in = nc.declare_dram_parameter("b3", [128, OUT_C], F32R, isOutput=False)
    out = nc.declare_dram_parameter("out", [B, OUT_C, PIX_B], F32, isOutput=True)

    act_fun = AF.Gelu if act == "gelu" else AF.Tanh

    with tile.TileContext(nc) as tc:
        with (
            tc.tile_pool(name="wpool", bufs=1) as wpool,
            tc.tile_pool(name="xpool", bufs=2) as xpool,
            tc.tile_pool(name="gbpool", bufs=3) as gbpool,
            tc.tile_pool(name="hpool", bufs=6) as hpool,
            tc.tile_pool(name="gspool", bufs=3) as gspool,
            tc.tile_pool(name="psL1", bufs=2, space="PSUM") as psL1,
            tc.tile_pool(name="psL2", bufs=2, space="PSUM") as psL2,
            tc.tile_pool(name="psL3", bufs=2, space="PSUM") as psL3,
            tc.tile_pool(name="psG", bufs=2, space="PSUM") as psG,
        ):
            # ---- persistent constants (small, load first) ---------------
            b1_sb = wpool.tile([128, E * 3], F32)
            b2_sb = wpool.tile([128, E * 3], F32)
            b3_sb = wpool.tile([128, OUT_C], F32R)
            rwt_sb = wpool.tile([R_C, E], F32)
            rb_bc = wpool.tile([128, E], F32)
            ident = wpool.tile([128, 128], F32)
            nc.sync.dma_start(rwt_sb[:], rwt_in[:])
            nc.sync.dma_start(rb_bc[:], rb_in[:])
            make_identity(nc, ident[:])
            nc.gpsimd.dma_start(b1_sb[:], b1_in[:])
            nc.gpsimd.dma_start(b2_sb[:], b2_in[:])
            nc.gpsimd.dma_start(b3_sb[:], b3_in[:])

            # ---- per-expert weights (one tile per tensor per expert) ----
            w1h_sb, w1l_sb, w2_sb, w3_sb = [], [], [], []
            for e in range(E):
                w1h_e = wpool.tile([96, 2, HID], FP8, name=f"w1h_{e}")
                w1l_e = wpool.tile([96, 2, HID], FP8, name=f"w1l_{e}")
                w2_e = wpool.tile([128, 3, HID], F32R, name=f"w2_{e}")
                w3_e = wpool.tile([128, 3, OUT_C], F32R, name=f"w3_{e}")
                nc.gpsimd.dma_start(w1h_e[:], w1h_in[e])
                nc.gpsimd.dma_start(w1l_e[:], w1l_in[e])
                nc.gpsimd.dma_start(w2_e[:], w2_in[e])
                nc.gpsimd.dma_start(w3_e[:], w3_in[e])
                w1h_sb.append(w1h_e)
                w1l_sb.append(w1l_e)
                w2_sb.append(w2_e)
                w3_sb.append(w3_e)

            # ---- main loop ----------------------------------------------
            for b in range(B):
                x8h_sb = xpool.tile([96, 2, PIX_B], FP8, tag="xh")
                x8l_sb = xpool.tile([96, 2, PIX_B], FP8, tag="xl")
                r_sb = xpool.tile([R_C, PIX_B], F32, tag="r")
                nc.sync.dma_start(x8h_sb[:], x8h_in[b])
                nc.sync.dma_start(x8l_sb[:], x8l_in[b])
                nc.sync.dma_start(r_sb[:], r_in[b])

                for t in range(NT_B):
                    p0 = t * TILE

                    # ---- gates ------------------------------------------
                    g_sb = gspool.tile([128, TILE], F32R, tag="g_sb")
                    lt4_ps = psG.tile([128, 64], F32, tag="ps_g", name="lt4")
                    for s in range(TILE // 128):
                        nc.tensor.matmul(
                            lt4_ps[:, 16 * s : 16 * s + E],
                            r_sb[:, p0 + 128 * s : p0 + 128 * (s + 1)],
                            rwt_sb[:],
                            start=True,
                            stop=True,
                        )
                    gs4 = []
                    for s in range(TILE // 128):
                        lt = gspool.tile([128, E], F32, tag="lt")
                        nc.vector.tensor_add(
                            lt[:], lt4_ps[:, 16 * s : 16 * s + E], rb_bc[:]
                        )
                        m1 = gspool.tile([128, 1], F32, tag="m1")
                        nc.vector.tensor_reduce(
                            m1[:], lt[:], axis=mybir.AxisListType.X, op=ALU.max
                        )
                        eq1 = gspool.tile([128, E], F32, tag="eq1")
                        nc.vector.tensor_single_scalar(
                            eq1[:], lt[:], m1[:], ALU.is_equal
                        )
                        msk = gspool.tile([128, E], F32, tag="msk")
                        nc.vector.scalar_tensor_tensor(
                            msk[:], eq1[:], -1e30, lt[:], ALU.mult, ALU.add
                        )
                        m2 = gspool.tile([128, 1], F32, tag="m2")
                        nc.vector.tensor_reduce(
                            m2[:], msk[:], axis=mybir.AxisListType.X, op=ALU.max
                        )
                        d = gspool.tile([128, 1], F32, tag="d")
                        nc.vector.tensor_sub(d[:], m2[:], m1[:])
                        tg = gspool.tile([128, 1], F32, tag="tg")
                        nc.scalar.activation(tg[:], d[:], AF.Tanh, scale=0.5)
                        g2 = gspool.tile([128, 1], F32, tag="g2")
                        nc.vector.tensor_scalar(
                            g2[:], tg[:], 0.5, 0.5, ALU.mult, ALU.add
                        )
                        g1 = gspool.tile([128, 1], F32, tag="g1")
                        nc.vector.tensor_scalar(
                            g1[:], tg[:], -0.5, 0.5, ALU.mult, ALU.add
                        )
                        eq2 = gspool.tile([128, E], F32, tag="eq2")
                        nc.vector.tensor_single_scalar(
                            eq2[:], lt[:], m2[:], ALU.is_equal
                        )
                        gt2 = gspool.tile([128, E], F32, tag="gt2")
                        nc.vector.tensor_single_scalar(gt2[:], eq2[:], g2[:], ALU.mult)
                        gs = gspool.tile([128, 128], F32, tag="gs")
                        nc.vector.memset(gs[:], 0.0)
                        nc.vector.scalar_tensor_tensor(
                            gs[:, :E], eq1[:], g1[:], gt2[:], ALU.mult, ALU.add
                        )
                        gs4.append(gs)
                    gT4_ps = psG.tile([128, TILE], F32, tag="ps_g", name="gT4")
                    for s in range(TILE // 128):
                        nc.tensor.transpose(
                            gT4_ps[:, 128 * s : 128 * (s + 1)], gs4[s][:], ident[:]
                        )
                    nc.scalar.copy(g_sb[:], gT4_ps[:])

                    # ---- experts ----------------------------------------
                    o_ps0 = psL3.tile([128, TILE], F32, tag="ps_o", name="o_ps0")
                    o_ps1 = psL3.tile([128, TILE], F32, tag="ps_o", name="o_ps1")
                    o_ps = [o_ps0[:128], o_ps1[: OUT_C - 128]]
                    for e in range(E):
                        # partition_broadcast reads partition 0 of its input;
                        # stage gate row e there with a tiny SBUF->SBUF DMA.
                        grow = gbpool.tile([1, TILE], F32R, tag="grow")
                        nc.sync.dma_start(grow[:], g_sb[e : e + 1, :])
                        gb = gbpool.tile([128, TILE], F32R, tag="gb")
                        nc.gpsimd.partition_broadcast(gb[:], grow[:])

                        h1 = []
                        for m in range(3):
                            ps1 = psL1.tile([128, TILE], F32, tag="ps1")
                            # L1 in fp8 DoubleRow (K=2x96 per instr), three
                            # error-compensation terms accumulated in PSUM:
                            # 16*h1pre = Wh@xh + Wh@xl + Wl@xh  (W prescaled
                            # x16 on host; ACT descales via scale=1/16).
                            for w_sb, x_t, st, sp in (
                                (w1h_sb[e], x8h_sb, True, False),
                                (w1h_sb[e], x8l_sb, False, False),
                                (w1l_sb[e], x8h_sb, False, True),
                            ):
                                nc.tensor.matmul(
                                    ps1[:],
                                    w_sb[:, :, 128 * m : 128 * (m + 1)],
                                    x_t[:, :, p0 : p0 + TILE],
                                    start=st,
                                    stop=sp,
                                    perf_mode=DR,
                                )
                            h1_m = hpool.tile([128, TILE], F32R, tag="h1")
                            nc.scalar.activation(
                                h1_m[:],
                                ps1[:],
                                act_fun,
                                bias=b1_sb[:, 3 * e + m : 3 * e + m + 1],
                                scale=1.0 / 16.0,
                            )
                            h1.append(h1_m)

                        h2 = []
                        for m in range(3):
                            ps2 = psL2.tile([128, TILE], F32, tag="ps2")
                            for k in range(3):
                                nc.tensor.matmul(
                                    ps2[:],
                                    w2_sb[e][:, k, 128 * m : 128 * (m + 1)],
                                    h1[k][:],
                                    start=(k == 0),
                                    stop=(k == 2),
                                )
                            h2_m = hpool.tile([128, TILE], F32R, tag="h2")
                            nc.scalar.activation(
                                h2_m[:],
                                ps2[:],
                                act_fun,
                                bias=b2_sb[:, 3 * e + m : 3 * e + m + 1],
                            )
                            nc.vector.tensor_mul(h2_m[:], h2_m[:], gb[:])
                            h2.append(h2_m)

                        for m, rows in ((0, 128), (1, OUT_C - 128)):
                            for k in range(3):
                                nc.tensor.matmul(
                                    o_ps[m][:],
                                    w3_sb[e][:, k, 128 * m : 128 * m + rows],
                                    h2[k][:],
                                    start=(e == 0 and k == 0),
                                    stop=False,
                                )

                    # b3 contribution: sum_e g_e * b3[e]  (K=8 matmul)
                    for m, rows in ((0, 128), (1, OUT_C - 128)):
                        nc.tensor.matmul(
                            o_ps[m][:],
                            b3_sb[:, 128 * m : 128 * m + rows],
                            g_sb[:],
                            start=False,
                            stop=True,
                        )
                        o_sb = hpool.tile([128, TILE], F32, tag="o_sb")
                        nc.vector.tensor_copy(o_sb[:rows], o_ps[m][:])
                        nc.sync.dma_start(
                            out[b, 128 * m : 128 * m + rows, p0 : p0 + TILE],
                            o_sb[:rows],
                        )

    if compile:
        nc.compile()
    return nc


def _get_nc(act: str = "gelu"):
    if act not in _nc_cache:
        _nc_cache[act] = _build(act)
    return _nc_cache[act]


def make_in_maps(x, router_input, router_W, router_b, W1, b1, W2, b2, W3, b3):
    f = np.float32
    fp8 = ml_dtypes.float8_e4m3
    # L1 weights: transpose to [c, h], prescale x16, split hi + residual
    # into fp8, pack contraction as 2 k-tiles of 96 for DoubleRow.
    w1t16 = np.transpose(np.asarray(W1, f), (0, 2, 1)) * 16.0  # [E,192,HID]
    w1h = w1t16.astype(fp8)
    w1l = (w1t16 - w1h.astype(f)).astype(fp8)
    w1h = np.ascontiguousarray(w1h.reshape(E, 2, 96, HID).transpose(0, 2, 1, 3))
    w1l = np.ascontiguousarray(w1l.reshape(E, 2, 96, HID).transpose(0, 2, 1, 3))
    w2t = np.transpose(np.asarray(W2, f), (0, 2, 1))
    w2t = np.ascontiguousarray(w2t.reshape(E, 3, 128, HID).transpose(0, 2, 1, 3))
    w3t = np.transpose(np.asarray(W3, f), (0, 2, 1))
    w3t = np.ascontiguousarray(w3t.reshape(E, 3, 128, OUT_C).transpose(0, 2, 1, 3))
    rwt = np.ascontiguousarray(np.asarray(router_W, f).T)
    rb = np.ascontiguousarray(np.tile(np.asarray(router_b, f).reshape(1, E), (128, 1)))
    b1t = np.ascontiguousarray(
        np.asarray(b1, f).reshape(E, 3, 128).transpose(2, 0, 1).reshape(128, E * 3)
    )
    b2t = np.ascontiguousarray(
        np.asarray(b2, f).reshape(E, 3, 128).transpose(2, 0, 1).reshape(128, E * 3)
    )
    b3a = np.zeros((128, OUT_C), f)
    b3a[:E] = np.asarray(b3, f)
    x = np.asarray(x, f)
    r = np.asarray(router_input, f)

    in_maps = []
    for c in range(N_CORES):
        h0 = c * HS
        xs = x[:, :, h0 : h0 + HS, :].reshape(B, IN_C, PIX_B)
        xh = xs.astype(fp8)
        xl = (xs - xh.astype(f)).astype(fp8)
        xh = np.ascontiguousarray(xh.reshape(B, 2, 96, PIX_B).transpose(0, 2, 1, 3))
        xl = np.ascontiguousarray(xl.reshape(B, 2, 96, PIX_B).transpose(0, 2, 1, 3))
        rs = np.ascontiguousarray(r[:, :, h0 : h0 + HS, :]).reshape(B, R_C, PIX_B)
        in_maps.append(
            {
                "x8h": xh,
                "x8l": xl,
                "r": rs,
                "w1h": w1h,
                "w1l": w1l,
                "w2t": w2t,
                "w3t": w3t,
                "rwt": rwt,
                "rb": rb,
                "b1t": b1t,
                "b2t": b2t,
                "b3": b3a,
            }
        )
    return in_maps


def kernel(x, router_input, router_W, router_b, W1, b1, W2, b2, W3, b3, **run_kwargs):
    nc = _get_nc("gelu")
    in_maps = make_in_maps(
        x, router_input, router_W, router_b, W1, b1, W2, b2, W3, b3
    )
    res = run_bass_kernel_spmd(nc, in_maps, list(range(N_CORES)), **run_kwargs)
    outs = [
        res.results[c]["out"].reshape(B, OUT_C, HS, W) for c in range(N_CORES)
    ]
    full = np.concatenate(outs, axis=2)
    if run_kwargs:
        kernel.last_results = res
    return full



# revision 26
# speedup vs baseline: 1.0534x; 1.0534x over previous
"""MoE update-MLP Trainium2 kernel (8-core SPMD, data-parallel over pixels).

Problem: x (4,192,128,128); a per-pixel router picks top-2 of 8 experts; each
expert is a 3-layer 1x1-conv MLP (192->384 gelu ->384 gelu ->192); output is
the gate-weighted sum over experts.

Sharding: H=128 split into 8 chunks of 16 rows; each core handles
4*16*128 = 8192 pixels and computes all 8 experts densely (gates of
non-top-2 experts are exactly 0, so dense gate-weighted accumulation is
exact, and per-pixel dynamic routing/gather is avoided).

Per 512-pixel tile on each core:
 - router logits computed transposed ([128 pix, 8 experts]) via K=8 fp32
   matmuls (fp32 so top-2 ranking matches the fp32 reference bit-for-bit)
 - top-2 + 2-way softmax via masked-max + is_equal on DVE and a
   tanh-based sigmoid on ACT (gelu_and_others table has Gelu+Tanh, so a
   single activation-table load covers the whole kernel)
 - gates transposed back with 4 PE transposes into one PSUM tile; each
   expert's gate row broadcast to [128, 512] on GpSimd with
   partition_broadcast (keeps those rows off the saturated PE array)
 - per expert: L1 (2x3 matmuls, contraction zero-padded 192->256 to keep
   K=128), exact Gelu+per-partition bias on ACT, L2 (3x3), Gelu+bias,
   per-pixel gate multiply on DVE, L3 accumulated over all 8 experts in
   PSUM; the b3 bias enters as one K=8-padded matmul against the gate
   rows (sum_e g_e*b3_e); result copied to SBUF on DVE and DMA'd out.

All matmul operands are float32r (full-rate on the PE at free-dim 512,
~1e-4 rounding). Weights are staged per-expert in SBUF tiles (one
contiguous DMA each, issued from GpSimd so the Sync sequencer's serial
descriptor generation doesn't delay the first tile's x/r loads).
"""

import numpy as np

import concourse.bacc as bacc
import concourse.mybir as mybir
import concourse.tile as tile
from concourse.bass_utils import run_bass_kernel_spmd
from concourse.masks import make_identity

F32 = mybir.dt.float32
F32R = mybir.dt.float32r
BF16 = mybir.dt.bfloat16
AF = mybir.ActivationFunctionType
ALU = mybir.AluOpType

N_CORES = 8
B, IN_C, H, W = 4, 192, 128, 128
R_C, E, HID, OUT_C = 8, 8, 384, 192
HS = H // N_CORES            # 16 rows of H per core
PIX_B = HS * W               # 2048 pixels per batch image per core
TILE = 512                   # pixels per compute tile
NT_B = PIX_B // TILE         # 4 tiles per batch image

_nc_cache: dict = {}


def _build(act: str = "gelu", compile: bool = True):
    """Build the (SPMD-identical) Bass program for one core."""
    nc = bacc.Bacc("TRN2", target_bir_lowering=False, debug=False)

    x_in = nc.declare_dram_parameter("x", [B, 256, PIX_B], F32R, isOutput=False)
    r_in = nc.declare_dram_parameter("r", [B, R_C, PIX_B], F32, isOutput=False)
    w1_in = nc.declare_dram_parameter("w1t", [E, 128, 2, HID], F32R, isOutput=False)
    w2_in = nc.declare_dram_parameter("w2t", [E, 128, 3, HID], F32R, isOutput=False)
    w3_in = nc.declare_dram_parameter("w3t", [E, 128, 3, OUT_C], F32R, isOutput=False)
    rwt_in = nc.declare_dram_parameter("rwt", [R_C, E], F32, isOutput=False)
    rb_in = nc.declare_dram_parameter("rb", [128, E], F32, isOutput=False)
    b1_in = nc.declare_dram_parameter("b1t", [128, E * 3], F32, isOutput=False)
    b2_in = nc.declare_dram_parameter("b2t", [128, E * 3], F32, isOutput=False)
    b3_in = nc.declare_dram_parameter("b3", [128, OUT_C], F32R, isOutput=False)
    out = nc.declare_dram_parameter("out", [B, OUT_C, PIX_B], F32, isOutput=True)

    act_fun = AF.Gelu if act == "gelu" else AF.Tanh

    with tile.TileContext(nc) as tc:
        with (
            tc.tile_pool(name="wpool", bufs=1) as wpool,
            tc.tile_pool(name="xpool", bufs=2) as xpool,
            tc.tile_pool(name="gbpool", bufs=3) as gbpool,
            tc.tile_pool(name="hpool", bufs=6) as hpool,
            tc.tile_pool(name="gspool", bufs=3) as gspool,
            tc.tile_pool(name="psL1", bufs=2, space="PSUM") as psL1,
            tc.tile_pool(name="psL2", bufs=2, space="PSUM") as psL2,
            tc.tile_pool(name="psL3", bufs=2, space="PSUM") as psL3,
            tc.tile_pool(name="psG", bufs=2, space="PSUM") as psG,
        ):
            # ---- persistent constants (small, load first) ---------------
            b1_sb = wpool.tile([128, E * 3], F32)
            b2_sb = wpool.tile([128, E * 3], F32)
            b3_sb = wpool.tile([128, OUT_C], F32R)
            rwt_sb = wpool.tile([R_C, E], F32)
            rb_bc = wpool.tile([128, E], F32)
            ident = wpool.tile([128, 128], BF16)
            nc.sync.dma_start(rwt_sb[:], rwt_in[:])
            nc.sync.dma_start(rb_bc[:], rb_in[:])
            make_identity(nc, ident[:])
            nc.gpsimd.dma_start(b1_sb[:], b1_in[:])
            nc.gpsimd.dma_start(b2_sb[:], b2_in[:])
            nc.gpsimd.dma_start(b3_sb[:], b3_in[:])

            # ---- per-expert weights (one tile per tensor per expert) ----
            w1_sb, w2_sb, w3_sb = [], [], []
            for e in range(E):
                w1_e = wpool.tile([128, 2, HID], F32R, name=f"w1_{e}")
                w2_e = wpool.tile([128, 3, HID], F32R, name=f"w2_{e}")
                w3_e = wpool.tile([128, 3, OUT_C], F32R, name=f"w3_{e}")
                # expert 0's weights ride the (otherwise idle) Sync queue so
                # the first tile's expert loop isn't stuck behind the serial
                # GpSimd descriptor generation for all 24 weight DMAs.
                dma_q = nc.sync if e == 0 else nc.gpsimd
                dma_q.dma_start(w1_e[:], w1_in[e])
                dma_q.dma_start(w2_e[:], w2_in[e])
                dma_q.dma_start(w3_e[:], w3_in[e])
                w1_sb.append(w1_e)
                w2_sb.append(w2_e)
                w3_sb.append(w3_e)

            # ---- main loop ----------------------------------------------
            for b in range(B):
                x_sb = xpool.tile([128, 2, PIX_B], F32R, tag="x")
                r_sb = xpool.tile([R_C, PIX_B], F32, tag="r")
                nc.sync.dma_start(x_sb[:, 0, :], x_in[b, 0:128, :])
                nc.sync.dma_start(x_sb[:, 1, :], x_in[b, 128:256, :])
                nc.sync.dma_start(r_sb[:], r_in[b])

                for t in range(NT_B):
                    p0 = t * TILE

                    # ---- gates ------------------------------------------
                    g_sb = gspool.tile([128, TILE], F32R, tag="g_sb")
                    lt4_ps = psG.tile([128, 64], F32, tag="ps_g", name="lt4")
                    for s in range(TILE // 128):
                        nc.tensor.matmul(
                            lt4_ps[:, 16 * s : 16 * s + E],
                            r_sb[:, p0 + 128 * s : p0 + 128 * (s + 1)],
                            rwt_sb[:],
                            start=True,
                            stop=True,
                        )
                    gs4 = []
                    for s in range(TILE // 128):
                        lt = gspool.tile([128, E], F32, tag="lt")
                        nc.vector.tensor_add(
                            lt[:], lt4_ps[:, 16 * s : 16 * s + E], rb_bc[:]
                        )
                        m1 = gspool.tile([128, 1], F32, tag="m1")
                        nc.vector.tensor_reduce(
                            m1[:], lt[:], axis=mybir.AxisListType.X, op=ALU.max
                        )
                        eq1 = gspool.tile([128, E], F32, tag="eq1")
                        nc.vector.tensor_single_scalar(
                            eq1[:], lt[:], m1[:], ALU.is_equal
                        )
                        msk = gspool.tile([128, E], F32, tag="msk")
                        nc.vector.scalar_tensor_tensor(
                            msk[:], eq1[:], -1e30, lt[:], ALU.mult, ALU.add
                        )
                        m2 = gspool.tile([128, 1], F32, tag="m2")
                        nc.vector.tensor_reduce(
                            m2[:], msk[:], axis=mybir.AxisListType.X, op=ALU.max
                        )
                        d = gspool.tile([128, 1], F32, tag="d")
                        nc.vector.tensor_sub(d[:], m2[:], m1[:])
                        tg = gspool.tile([128, 1], F32, tag="tg")
                        nc.scalar.activation(tg[:], d[:], AF.Tanh, scale=0.5)
                        g2 = gspool.tile([128, 1], F32, tag="g2")
                        nc.vector.tensor_scalar(
                            g2[:], tg[:], 0.5, 0.5, ALU.mult, ALU.add
                        )
                        g1 = gspool.tile([128, 1], F32, tag="g1")
                        nc.vector.tensor_scalar(
                            g1[:], tg[:], -0.5, 0.5, ALU.mult, ALU.add
                        )
                        eq2 = gspool.tile([128, E], F32, tag="eq2")
                        nc.vector.tensor_single_scalar(
                            eq2[:], lt[:], m2[:], ALU.is_equal
                        )
                        gt2 = gspool.tile([128, E], F32, tag="gt2")
                        nc.vector.tensor_single_scalar(gt2[:], eq2[:], g2[:], ALU.mult)
                        gs = gspool.tile([128, 128], BF16, tag="gs")
                        nc.vector.memset(gs[:], 0.0)
                        nc.vector.scalar_tensor_tensor(
                            gs[:, :E], eq1[:], g1[:], gt2[:], ALU.mult, ALU.add
                        )
                        gs4.append(gs)
                    gT4_ps = psG.tile([128, TILE], BF16, tag="ps_g", name="gT4")
                    for s in range(TILE // 128):
                        nc.tensor.transpose(
                            gT4_ps[:, 128 * s : 128 * (s + 1)], gs4[s][:], ident[:]
                        )
                    nc.scalar.copy(g_sb[:], gT4_ps[:])

                    # ---- experts ----------------------------------------
                    o_ps0 = psL3.tile([128, TILE], F32, tag="ps_o", name="o_ps0")
                    o_ps1 = psL3.tile([128, TILE], F32, tag="ps_o", name="o_ps1")
                    o_ps = [o_ps0[:128], o_ps1[: OUT_C - 128]]
                    for e in range(E):
                        # partition_broadcast reads partition 0 of its input;
                        # stage gate row e there with a tiny SBUF->SBUF DMA.
                        grow = gbpool.tile([1, TILE], F32R, tag="grow")
                        nc.sync.dma_start(grow[:], g_sb[e : e + 1, :])
                        gb = gbpool.tile([128, TILE], F32R, tag="gb")
                        nc.gpsimd.partition_broadcast(gb[:], grow[:])

                        h1 = []
                        for m in range(3):
                            ps1 = psL1.tile([128, TILE], F32, tag="ps1")
                            nc.tensor.matmul(
                                ps1[:],
                                w1_sb[e][:, 0, 128 * m : 128 * (m + 1)],
                                x_sb[:, 0, p0 : p0 + TILE],
                                start=True,
                                stop=False,
                            )
                            nc.tensor.matmul(
                                ps1[:],
                                w1_sb[e][:, 1, 128 * m : 128 * (m + 1)],
                                x_sb[:, 1, p0 : p0 + TILE],
                                start=False,
                                stop=True,
                            )
                            h1_m = hpool.tile([128, TILE], F32R, tag="h1")
                            nc.scalar.activation(
                                h1_m[:],
                                ps1[:],
                                act_fun,
                                bias=b1_sb[:, 3 * e + m : 3 * e + m + 1],
                            )
                            h1.append(h1_m)

                        h2 = []
                        for m in range(3):
                            ps2 = psL2.tile([128, TILE], F32, tag="ps2")
                            for k in range(3):
                                nc.tensor.matmul(
                                    ps2[:],
                                    w2_sb[e][:, k, 128 * m : 128 * (m + 1)],
                                    h1[k][:],
                                    start=(k == 0),
                                    stop=(k == 2),
                                )
                            h2_m = hpool.tile([128, TILE], F32R, tag="h2")
                            nc.scalar.activation(
                                h2_m[:],
                                ps2[:],
                                act_fun,
                                bias=b2_sb[:, 3 * e + m : 3 * e + m + 1],
                            )
                            nc.vector.tensor_mul(h2_m[:], h2_m[:], gb[:])
                            h2.append(h2_m)

                        for m, rows in ((0, 128), (1, OUT_C - 128)):
                            for k in range(3):
                                nc.tensor.matmul(
                                    o_ps[m][:],
                                    w3_sb[e][:, k, 128 * m : 128 * m + rows],
                                    h2[k][:],
                                    start=(e == 0 and k == 0),
                                    stop=False,
                                )

                    # b3 contribution: sum_e g_e * b3[e]  (K=8 matmul)
                    for m, rows in ((0, 128), (1, OUT_C - 128)):
                        nc.tensor.matmul(
                            o_ps[m][:],
                            b3_sb[:, 128 * m : 128 * m + rows],
                            g_sb[:],
                            start=False,
                            stop=True,
                        )
                        o_sb = hpool.tile([128, TILE], F32, tag="o_sb")
                        nc.vector.tensor_copy(o_sb[:rows], o_ps[m][:])
                        nc.sync.dma_start(
                            out[b, 128 * m : 128 * m + rows, p0 : p0 + TILE],
                            o_sb[:rows],
                        )

    if compile:
        nc.compile()
    return nc


def _get_nc(act: str = "gelu"):
    if act not in _nc_cache:
        _nc_cache[act] = _build(act)
    return _nc_cache[act]


def make_in_maps(x, router_input, router_W, router_b, W1, b1, W2, b2, W3, b3):
    f = np.float32
    w1t = np.zeros((E, 256, HID), f)
    w1t[:, :IN_C, :] = np.transpose(np.asarray(W1, f), (0, 2, 1))
    w1t = np.ascontiguousarray(w1t.reshape(E, 2, 128, HID).transpose(0, 2, 1, 3))
    w2t = np.transpose(np.asarray(W2, f), (0, 2, 1))
    w2t = np.ascontiguousarray(w2t.reshape(E, 3, 128, HID).transpose(0, 2, 1, 3))
    w3t = np.transpose(np.asarray(W3, f), (0, 2, 1))
    w3t = np.ascontiguousarray(w3t.reshape(E, 3, 128, OUT_C).transpose(0, 2, 1, 3))
    rwt = np.ascontiguousarray(np.asarray(router_W, f).T)
    rb = np.ascontiguousarray(np.tile(np.asarray(router_b, f).reshape(1, E), (128, 1)))
    b1t = np.ascontiguousarray(
        np.asarray(b1, f).reshape(E, 3, 128).transpose(2, 0, 1).reshape(128, E * 3)
    )
    b2t = np.ascontiguousarray(
        np.asarray(b2, f).reshape(E, 3, 128).transpose(2, 0, 1).reshape(128, E * 3)
    )
    b3a = np.zeros((128, OUT_C), f)
    b3a[:E] = np.asarray(b3, f)
    x = np.asarray(x, f)
    r = np.asarray(router_input, f)

    in_maps = []
    for c in range(N_CORES):
        h0 = c * HS
        xs = np.zeros((B, 256, PIX_B), f)
        xs[:, :IN_C] = x[:, :, h0 : h0 + HS, :].reshape(B, IN_C, PIX_B)
        rs = np.ascontiguousarray(r[:, :, h0 : h0 + HS, :]).reshape(B, R_C, PIX_B)
        in_maps.append(
            {
                "x": xs,
                "r": rs,
                "w1t": w1t,
                "w2t": w2t,
                "w3t": w3t,
                "rwt": rwt,
                "rb": rb,
                "b1t": b1t,
                "b2t": b2t,
                "b3": b3a,
            }
        )
    return in_maps


def kernel(x, router_input, router_W, router_b, W1, b1, W2, b2, W3, b3, **run_kwargs):
    nc = _get_nc("gelu")
    in_maps = make_in_maps(
        x, router_input, router_W, router_b, W1, b1, W2, b2, W3, b3
    )
    res = run_bass_kernel_spmd(nc, in_maps, list(range(N_CORES)), **run_kwargs)
    outs = [
        res.results[c]["out"].reshape(B, OUT_C, HS, W) for c in range(N_CORES)
    ]
    full = np.concatenate(outs, axis=2)
    if run_kwargs:
        kernel.last_results = res
    return full



# revision 30
# speedup vs baseline: 1.0638x; 1.0099x over previous
"""MoE update-MLP Trainium2 kernel (8-core SPMD, data-parallel over pixels).

Problem: x (4,192,128,128); a per-pixel router picks top-2 of 8 experts; each
expert is a 3-layer 1x1-conv MLP (192->384 gelu ->384 gelu ->192); output is
the gate-weighted sum over experts.

Sharding: H=128 split into 8 chunks of 16 rows; each core handles
4*16*128 = 8192 pixels and computes all 8 experts densely (gates of
non-top-2 experts are exactly 0, so dense gate-weighted accumulation is
exact, and per-pixel dynamic routing/gather is avoided).

Per 512-pixel tile on each core:
 - router logits computed transposed ([128 pix, 8 experts]) via K=8 fp32
   matmuls (fp32 so top-2 ranking matches the fp32 reference bit-for-bit)
 - top-2 + 2-way softmax via masked-max + is_equal on DVE and a
   tanh-based sigmoid on ACT (gelu_and_others table has Gelu+Tanh, so a
   single activation-table load covers the whole kernel)
 - gates transposed back with 4 PE transposes into one PSUM tile; each
   expert's gate row broadcast to [128, 512] on GpSimd with
   partition_broadcast (keeps those rows off the saturated PE array)
 - per expert: L1 (2x3 matmuls, contraction zero-padded 192->256 to keep
   K=128), exact Gelu+per-partition bias on ACT, L2 (3x3), Gelu+bias,
   per-pixel gate multiply on DVE, L3 accumulated over all 8 experts in
   PSUM; the b3 bias enters as one K=8-padded matmul against the gate
   rows (sum_e g_e*b3_e); result copied to SBUF on DVE and DMA'd out.

All matmul operands are float32r (full-rate on the PE at free-dim 512,
~1e-4 rounding). Weights are staged per-expert in SBUF tiles (one
contiguous DMA each, issued from GpSimd so the Sync sequencer's serial
descriptor generation doesn't delay the first tile's x/r loads).
"""

import numpy as np

import concourse.bacc as bacc
import concourse.mybir as mybir
import concourse.tile as tile
from concourse.bass_utils import run_bass_kernel_spmd
from concourse.masks import make_identity

F32 = mybir.dt.float32
F32R = mybir.dt.float32r
AF = mybir.ActivationFunctionType
ALU = mybir.AluOpType

N_CORES = 8
B, IN_C, H, W = 4, 192, 128, 128
R_C, E, HID, OUT_C = 8, 8, 384, 192
HS = H // N_CORES            # 16 rows of H per core
PIX_B = HS * W               # 2048 pixels per batch image per core
TILE = 512                   # pixels per compute tile
NT_B = PIX_B // TILE         # 4 tiles per batch image

_nc_cache: dict = {}


def _build(act: str = "gelu", compile: bool = True):
    """Build the (SPMD-identical) Bass program for one core."""
    nc = bacc.Bacc("TRN2", target_bir_lowering=False, debug=False)

    x_in = nc.declare_dram_parameter("x", [B, 256, PIX_B], F32R, isOutput=False)
    r_in = nc.declare_dram_parameter("r", [B, R_C, PIX_B], F32, isOutput=False)
    w1_in = nc.declare_dram_parameter("w1t", [E, 128, 2, HID], F32R, isOutput=False)
    w2_in = nc.declare_dram_parameter("w2t", [E, 128, 3, HID], F32R, isOutput=False)
    w3_in = nc.declare_dram_parameter("w3t", [E, 128, 3, OUT_C], F32R, isOutput=False)
    rwt_in = nc.declare_dram_parameter("rwt", [R_C, E], F32, isOutput=False)
    rb_in = nc.declare_dram_parameter("rb", [128, E], F32, isOutput=False)
    b1_in = nc.declare_dram_parameter("b1t", [128, E * 3], F32, isOutput=False)
    b2_in = nc.declare_dram_parameter("b2t", [128, E * 3], F32, isOutput=False)
    b3_in = nc.declare_dram_parameter("b3", [128, OUT_C], F32R, isOutput=False)
    out = nc.declare_dram_parameter("out", [B, OUT_C, PIX_B], F32, isOutput=True)

    act_fun = AF.Gelu if act == "gelu" else AF.Tanh

    with tile.TileContext(nc) as tc:
        with (
            tc.tile_pool(name="wpool", bufs=1) as wpool,
            tc.tile_pool(name="xpool", bufs=2) as xpool,
            tc.tile_pool(name="gbpool", bufs=3) as gbpool,
            tc.tile_pool(name="hpool", bufs=6) as hpool,
            tc.tile_pool(name="gspool", bufs=3) as gspool,
            tc.tile_pool(name="psL1", bufs=2, space="PSUM") as psL1,
            tc.tile_pool(name="psL2", bufs=2, space="PSUM") as psL2,
            tc.tile_pool(name="psL3", bufs=2, space="PSUM") as psL3,
            tc.tile_pool(name="psG", bufs=2, space="PSUM") as psG,
        ):
            # ---- persistent constants (small, load first) ---------------
            b1_sb = wpool.tile([128, E * 3], F32)
            b2_sb = wpool.tile([128, E * 3], F32)
            b3_sb = wpool.tile([128, OUT_C], F32R)
            rwt_sb = wpool.tile([R_C, E], F32)
            rb_bc = wpool.tile([128, E], F32)
            ident = wpool.tile([128, 128], F32)
            nc.sync.dma_start(rwt_sb[:], rwt_in[:])
            nc.sync.dma_start(rb_bc[:], rb_in[:])
            make_identity(nc, ident[:])
            nc.gpsimd.dma_start(b1_sb[:], b1_in[:])
            nc.gpsimd.dma_start(b2_sb[:], b2_in[:])
            nc.gpsimd.dma_start(b3_sb[:], b3_in[:])

            # ---- per-expert weights (one tile per tensor per expert) ----
            # All on the GpSimd queue, but ordered by demand: every w1
            # first (tile 0 touches all eight L1s early), then per-expert
            # (w2_e, w3_e) pairs, which track the expert loop's pace.
            w1_sb, w2_sb, w3_sb = [], [], []
            for e in range(E):
                w1_e = wpool.tile([128, 2, HID], F32R, name=f"w1_{e}")
                w2_e = wpool.tile([128, 3, HID], F32R, name=f"w2_{e}")
                w3_e = wpool.tile([128, 3, OUT_C], F32R, name=f"w3_{e}")
                nc.gpsimd.dma_start(w1_e[:], w1_in[e])
                w1_sb.append(w1_e)
                w2_sb.append(w2_e)
                w3_sb.append(w3_e)
            for e in range(E):
                nc.gpsimd.dma_start(w2_sb[e][:], w2_in[e])
                nc.gpsimd.dma_start(w3_sb[e][:], w3_in[e])

            # ---- main loop ----------------------------------------------
            # Gates for tile t+1 are computed during tile t's expert loop
            # (PE logits + DVE top-2 chain emitted after expert 1, PE
            # transposes after expert 4) so the array never stalls on the
            # DVE softmax chain at tile boundaries.
            def emit_gates_a(r_sb, p0):
                """Router logits (PE) + top-2/softmax chain (DVE/ACT)."""
                lt4_ps = psG.tile([128, 64], F32, tag="ps_g", name="lt4")
                for s in range(TILE // 128):
                    nc.tensor.matmul(
                        lt4_ps[:, 16 * s : 16 * s + E],
                        r_sb[:, p0 + 128 * s : p0 + 128 * (s + 1)],
                        rwt_sb[:],
                        start=True,
                        stop=True,
                    )
                gs4 = []
                for s in range(TILE // 128):
                    lt = gspool.tile([128, E], F32, tag="lt")
                    nc.vector.tensor_add(
                        lt[:], lt4_ps[:, 16 * s : 16 * s + E], rb_bc[:]
                    )
                    m1 = gspool.tile([128, 1], F32, tag="m1")
                    nc.vector.tensor_reduce(
                        m1[:], lt[:], axis=mybir.AxisListType.X, op=ALU.max
                    )
                    eq1 = gspool.tile([128, E], F32, tag="eq1")
                    nc.vector.tensor_single_scalar(
                        eq1[:], lt[:], m1[:], ALU.is_equal
                    )
                    msk = gspool.tile([128, E], F32, tag="msk")
                    nc.vector.scalar_tensor_tensor(
                        msk[:], eq1[:], -1e30, lt[:], ALU.mult, ALU.add
                    )
                    m2 = gspool.tile([128, 1], F32, tag="m2")
                    nc.vector.tensor_reduce(
                        m2[:], msk[:], axis=mybir.AxisListType.X, op=ALU.max
                    )
                    d = gspool.tile([128, 1], F32, tag="d")
                    nc.vector.tensor_sub(d[:], m2[:], m1[:])
                    tg = gspool.tile([128, 1], F32, tag="tg")
                    nc.scalar.activation(tg[:], d[:], AF.Tanh, scale=0.5)
                    g2 = gspool.tile([128, 1], F32, tag="g2")
                    nc.vector.tensor_scalar(
                        g2[:], tg[:], 0.5, 0.5, ALU.mult, ALU.add
                    )
                    g1 = gspool.tile([128, 1], F32, tag="g1")
                    nc.vector.tensor_scalar(
                        g1[:], tg[:], -0.5, 0.5, ALU.mult, ALU.add
                    )
                    eq2 = gspool.tile([128, E], F32, tag="eq2")
                    nc.vector.tensor_single_scalar(
                        eq2[:], lt[:], m2[:], ALU.is_equal
                    )
                    gt2 = gspool.tile([128, E], F32, tag="gt2")
                    nc.vector.tensor_single_scalar(gt2[:], eq2[:], g2[:], ALU.mult)
                    gs = gspool.tile([128, 128], F32, tag="gs")
                    nc.vector.memset(gs[:], 0.0)
                    nc.vector.scalar_tensor_tensor(
                        gs[:, :E], eq1[:], g1[:], gt2[:], ALU.mult, ALU.add
                    )
                    gs4.append(gs)
                return gs4

            def emit_gates_b(gs4):
                """Transpose gates to [E, pix] layout and copy to SBUF."""
                g_sb = gspool.tile([128, TILE], F32R, tag="g_sb")
                gT4_ps = psG.tile([128, TILE], F32, tag="ps_g", name="gT4")
                for s in range(TILE // 128):
                    nc.tensor.transpose(
                        gT4_ps[:, 128 * s : 128 * (s + 1)], gs4[s][:], ident[:]
                    )
                nc.scalar.copy(g_sb[:], gT4_ps[:])
                return g_sb

            def emit_batch_dmas(b):
                # r first (the router only needs 64 KB to start tile 0);
                # x in per-tile quarters so tile deps are fine-grained.
                x_sb = xpool.tile([128, 2, PIX_B], F32R, tag="x")
                r_sb = xpool.tile([R_C, PIX_B], F32, tag="r")
                nc.sync.dma_start(r_sb[:], r_in[b])
                for q in range(NT_B):
                    qs = slice(q * TILE, (q + 1) * TILE)
                    nc.sync.dma_start(x_sb[:, 0, qs], x_in[b, 0:128, qs])
                    nc.sync.dma_start(x_sb[:, 1, qs], x_in[b, 128:256, qs])
                return x_sb, r_sb

            x_sb, r_sb = emit_batch_dmas(0)
            g_sb = emit_gates_b(emit_gates_a(r_sb, 0))
            for b in range(B):
                for t in range(NT_B):
                    p0 = t * TILE
                    nxt = None
                    g_next = None
                    x_nx = r_nx = None

                    # ---- experts ----------------------------------------
                    o_ps0 = psL3.tile([128, TILE], F32, tag="ps_o", name="o_ps0")
                    o_ps1 = psL3.tile([128, TILE], F32, tag="ps_o", name="o_ps1")
                    o_ps = [o_ps0[:128], o_ps1[: OUT_C - 128]]
                    for e in range(E):
                        # partition_broadcast reads partition 0 of its input;
                        # stage gate row e there with a tiny SBUF->SBUF DMA.
                        grow = gbpool.tile([1, TILE], F32R, tag="grow")
                        nc.sync.dma_start(grow[:], g_sb[e : e + 1, :])
                        gb = gbpool.tile([128, TILE], F32R, tag="gb")
                        nc.gpsimd.partition_broadcast(gb[:], grow[:])

                        h1 = []
                        for m in range(3):
                            ps1 = psL1.tile([128, TILE], F32, tag="ps1")
                            nc.tensor.matmul(
                                ps1[:],
                                w1_sb[e][:, 0, 128 * m : 128 * (m + 1)],
                                x_sb[:, 0, p0 : p0 + TILE],
                                start=True,
                                stop=False,
                            )
                            nc.tensor.matmul(
                                ps1[:],
                                w1_sb[e][:, 1, 128 * m : 128 * (m + 1)],
                                x_sb[:, 1, p0 : p0 + TILE],
                                start=False,
                                stop=True,
                            )
                            h1_m = hpool.tile([128, TILE], F32R, tag="h1")
                            nc.scalar.activation(
                                h1_m[:],
                                ps1[:],
                                act_fun,
                                bias=b1_sb[:, 3 * e + m : 3 * e + m + 1],
                            )
                            h1.append(h1_m)

                        h2 = []
                        for m in range(3):
                            ps2 = psL2.tile([128, TILE], F32, tag="ps2")
                            for k in range(3):
                                nc.tensor.matmul(
                                    ps2[:],
                                    w2_sb[e][:, k, 128 * m : 128 * (m + 1)],
                                    h1[k][:],
                                    start=(k == 0),
                                    stop=(k == 2),
                                )
                            h2_m = hpool.tile([128, TILE], F32R, tag="h2")
                            nc.scalar.activation(
                                h2_m[:],
                                ps2[:],
                                act_fun,
                                bias=b2_sb[:, 3 * e + m : 3 * e + m + 1],
                            )
                            nc.vector.tensor_mul(h2_m[:], h2_m[:], gb[:])
                            h2.append(h2_m)

                        for m, rows in ((0, 128), (1, OUT_C - 128)):
                            for k in range(3):
                                nc.tensor.matmul(
                                    o_ps[m][:],
                                    w3_sb[e][:, k, 128 * m : 128 * m + rows],
                                    h2[k][:],
                                    start=(e == 0 and k == 0),
                                    stop=False,
                                )

                        # ---- pipeline next tile's gates -----------------
                        if e == 1:
                            if t < NT_B - 1:
                                nxt = emit_gates_a(r_sb, p0 + TILE)
                            elif b < B - 1:
                                x_nx, r_nx = emit_batch_dmas(b + 1)
                                nxt = emit_gates_a(r_nx, 0)
                        if e == 4 and nxt is not None:
                            g_next = emit_gates_b(nxt)

                    # b3 contribution: sum_e g_e * b3[e]  (K=8 matmul)
                    for m, rows in ((0, 128), (1, OUT_C - 128)):
                        nc.tensor.matmul(
                            o_ps[m][:],
                            b3_sb[:, 128 * m : 128 * m + rows],
                            g_sb[:],
                            start=False,
                            stop=True,
                        )
                        o_sb = hpool.tile([128, TILE], F32, tag="o_sb")
                        nc.vector.tensor_copy(o_sb[:rows], o_ps[m][:])
                        nc.sync.dma_start(
                            out[b, 128 * m : 128 * m + rows, p0 : p0 + TILE],
                            o_sb[:rows],
                        )

                    if g_next is not None:
                        g_sb = g_next
                    if x_nx is not None:
                        x_sb, r_sb = x_nx, r_nx

    if compile:
        nc.compile()
    return nc


def _get_nc(act: str = "gelu"):
    if act not in _nc_cache:
        _nc_cache[act] = _build(act)
    return _nc_cache[act]


def make_in_maps(x, router_input, router_W, router_b, W1, b1, W2, b2, W3, b3):
    f = np.float32
    w1t = np.zeros((E, 256, HID), f)
    w1t[:, :IN_C, :] = np.transpose(np.asarray(W1, f), (0, 2, 1))
    w1t = np.ascontiguousarray(w1t.reshape(E, 2, 128, HID).transpose(0, 2, 1, 3))
    w2t = np.transpose(np.asarray(W2, f), (0, 2, 1))
    w2t = np.ascontiguousarray(w2t.reshape(E, 3, 128, HID).transpose(0, 2, 1, 3))
    w3t = np.transpose(np.asarray(W3, f), (0, 2, 1))
    w3t = np.ascontiguousarray(w3t.reshape(E, 3, 128, OUT_C).transpose(0, 2, 1, 3))
    rwt = np.ascontiguousarray(np.asarray(router_W, f).T)
    rb = np.ascontiguousarray(np.tile(np.asarray(router_b, f).reshape(1, E), (128, 1)))
    b1t = np.ascontiguousarray(
        np.asarray(b1, f).reshape(E, 3, 128).transpose(2, 0, 1).reshape(128, E * 3)
    )
    b2t = np.ascontiguousarray(
        np.asarray(b2, f).reshape(E, 3, 128).transpose(2, 0, 1).reshape(128, E * 3)
    )
    b3a = np.zeros((128, OUT_C), f)
    b3a[:E] = np.asarray(b3, f)
    x = np.asarray(x, f)
    r = np.asarray(router_input, f)

    in_maps = []
    for c in range(N_CORES):
        h0 = c * HS
        xs = np.zeros((B, 256, PIX_B), f)
        xs[:, :IN_C] = x[:, :, h0 : h0 + HS, :].reshape(B, IN_C, PIX_B)
        rs = np.ascontiguousarray(r[:, :, h0 : h0 + HS, :]).reshape(B, R_C, PIX_B)
        in_maps.append(
            {
                "x": xs,
                "r": rs,
                "w1t": w1t,
                "w2t": w2t,
                "w3t": w3t,
                "rwt": rwt,
                "rb": rb,
                "b1t": b1t,
                "b2t": b2t,
                "b3": b3a,
            }
        )
    return in_maps


def kernel(x, router_input, router_W, router_b, W1, b1, W2, b2, W3, b3, **run_kwargs):
    nc = _get_nc("gelu")
    in_maps = make_in_maps(
        x, router_input, router_W, router_b, W1, b1, W2, b2, W3, b3
    )
    res = run_bass_kernel_spmd(nc, in_maps, list(range(N_CORES)), **run_kwargs)
    outs = [
        res.results[c]["out"].reshape(B, OUT_C, HS, W) for c in range(N_CORES)
    ]
    full = np.concatenate(outs, axis=2)
    if run_kwargs:
        kernel.last_results = res
    return full

